# revision 13
# baseline (speedup 1.0000x reference)
"""Causal multi-head attention on 8 TRN2 NeuronCores.

Problem: B=4, H=16, S=2048, D=128 fp32 causal attention.
Sharding: batch*heads (64) split 8-per-core across the 8 cores; each core
computes its heads fully independently (no collectives).

Per-core kernel strategy (bf16 matmuls, f32 accumulation):
  - scores computed TRANSPOSED: S^T[k,q] = K_j @ Q^T per (k-block j of 128,
    q-group g of 512), causal blocks only
  - exp on ScalarE (PSUM -> SBUF bf16), 1/sqrt(D) folded into activation
    scale; diagonal 128x128 blocks masked with a constant triangular tile
  - PV: out[q,:] = P^T_slice.T @ [V_j | ones]; the appended ones column
    yields the softmax denominator in the same accumulation. Output lands
    directly in [q, d] layout.
  - normalize with VectorE reciprocal + per-partition tensor_scalar mult
"""

import numpy as np
import ml_dtypes

B, H, S, D = 4, 16, 2048, 128
N_CORES = 8
HPC = (B * H) // N_CORES  # heads per core = 8
QB = 128                  # q/k block
GW = 512                  # q group width
NG = S // GW              # 4 groups per head
NJ = S // QB              # 16 k blocks
VW = D + 1                # V augmented with ones column = 129
SCALE = 1.0 / float(np.sqrt(D))

_BF16 = ml_dtypes.bfloat16

_CACHE = {}


def _build():
    import concourse.bass as bass  # noqa: F401
    import concourse.mybir as mybir
    from concourse import bacc
    from concourse.tile import TileContext

    f32 = mybir.dt.float32
    bf16 = mybir.dt.bfloat16
    EXP = mybir.ActivationFunctionType.Exp

    nc = bacc.Bacc("TRN2", target_bir_lowering=False, num_devices=N_CORES)

    qt_d = nc.dram_tensor("qt", [HPC, 128, S], bf16, kind="ExternalInput").ap()
    kt_d = nc.dram_tensor("kt", [HPC, 128, S], bf16, kind="ExternalInput").ap()
    va_d = nc.dram_tensor("va", [HPC, 128, NJ * VW], bf16, kind="ExternalInput").ap()
    tri_d = nc.dram_tensor("tri", [128, 128], bf16, kind="ExternalInput").ap()
    out_d = nc.dram_tensor("out", [HPC, S, D], f32, kind="ExternalOutput").ap()

    with TileContext(nc) as tc:
        with (
            tc.tile_pool(name="consts", bufs=1) as consts,
            tc.tile_pool(name="io", bufs=2) as io,
            tc.tile_pool(name="pt", bufs=2) as ptp,
            tc.tile_pool(name="ob", bufs=3) as obp,
            tc.tile_pool(name="rr", bufs=4) as rrp,
            tc.tile_pool(name="st", bufs=2, space="PSUM") as stp,
            tc.tile_pool(name="acc", bufs=2, space="PSUM") as accp,
        ):
            tri_sb = consts.tile([128, 128], bf16)

            def load_head(h):
                qt_sb = io.tile([128, S], bf16, tag="qt", name=f"qt{h}")
                kt_sb = io.tile([128, S], bf16, tag="kt", name=f"kt{h}")
                va_sb = io.tile([128, NJ * VW], bf16, tag="va", name=f"va{h}")
                m = (NJ * VW) // 2
                nc.sync.dma_start(out=qt_sb[:, 0:GW], in_=qt_d[h, :, 0:GW])
                nc.sync.dma_start(out=kt_sb[:, 0:GW], in_=kt_d[h, :, 0:GW])
                if h == 0:
                    nc.sync.dma_start(out=tri_sb[:, :], in_=tri_d[:, :])
                for p in range(1, 4):
                    lo, hi = p * GW, (p + 1) * GW
                    nc.sync.dma_start(out=qt_sb[:, lo:hi], in_=qt_d[h, :, lo:hi])
                    nc.sync.dma_start(out=kt_sb[:, lo:hi], in_=kt_d[h, :, lo:hi])
                nc.sync.dma_start(out=va_sb[:, 0:m], in_=va_d[h, :, 0:m])
                nc.sync.dma_start(out=va_sb[:, m:], in_=va_d[h, :, m:])
                return qt_sb, kt_sb, va_sb

            def s_chunks(u):
                """Per chunk of unit u: (mm_closures_with_cost, exp_closure,
                act_cost). S^T matmuls land in bank-aligned PSUM chunks
                (<=3 banks), one exp per chunk, diag masks after the exp."""
                h, g, bufs, pt_sb, offs, chunks = u
                qt_sb, kt_sb, _ = bufs
                qhi = GW * (g + 1)
                for ci, (col0, entries, cw) in enumerate(chunks):
                    st = stp.tile([128, 1536], f32, tag="st",
                                  name=f"st{h}g{g}c{ci}")
                    mms = []
                    mmcost = 0
                    for (j, qlo, off, w) in entries:
                        def mm(j=j, qlo=qlo, off=off, w=w, st=st):
                            nc.tensor.matmul(
                                st[:, off:off + w],
                                lhsT=kt_sb[:, j * QB:(j + 1) * QB],
                                rhs=qt_sb[:, qlo:qhi],
                                start=True, stop=True,
                            )
                        mms.append(mm)
                        mmcost += w + 8

                    def ex(col0=col0, cw=cw, st=st, entries=entries):
                        nc.scalar.activation(
                            pt_sb[:, col0:col0 + cw], st[:, 0:cw], EXP,
                            scale=SCALE,
                        )
                        for (j, qlo, off, w) in entries:
                            if j >= 4 * g:  # diag block: zero where k > q
                                nc.vector.tensor_mul(
                                    pt_sb[:, col0 + off:col0 + off + QB],
                                    pt_sb[:, col0 + off:col0 + off + QB],
                                    tri_sb[:, :],
                                )
                    yield mms, mmcost, ex, 2 * (cw + 222)

            def pv_steps(u):
                """(pe_cost, closure) steps: PV accumulation matmuls +
                normalize + store for unit u."""
                h, g, bufs, pt_sb, offs, _chunks = u
                _, _, va_sb = bufs
                o_sb = obp.tile([128, GW], f32, tag="ob", name=f"ob{h}g{g}")
                for c in range(4):
                    Q = 4 * g + c
                    qlo_c = GW * g + QB * c
                    acc = accp.tile([128, VW], f32, tag="acc", name=f"acc{h}g{g}c{c}")
                    for j in range(Q + 1):
                        qlo_j, col_j = offs[j]
                        off = col_j + (qlo_c - qlo_j)

                        def step(j=j, Q=Q, off=off, acc=acc):
                            nc.tensor.matmul(
                                acc[:, :],
                                lhsT=pt_sb[:, off:off + QB],
                                rhs=va_sb[:, j * VW:(j + 1) * VW],
                                start=(j == 0), stop=(j == Q),
                            )
                        yield 135, step

                    def fin(c=c, acc=acc, o_sb=o_sb):
                        r = rrp.tile([128, 1], f32, tag="r", name=f"r{h}g{g}c{c}")
                        nc.vector.reciprocal(r[:, :], acc[:, D:D + 1])
                        nc.vector.tensor_scalar_mul(
                            o_sb[:, c * QB:(c + 1) * QB], acc[:, 0:D], r[:, :]
                        )
                        if c == 3:
                            dst = out_d[h, GW * g:GW * (g + 1), :].rearrange(
                                "(c p) d -> p c d", p=128
                            )
                            src = o_sb[:, :].rearrange("p (c d) -> p c d", c=4)
                            nc.sync.dma_start(out=dst, in_=src)
                    yield 0, fin

            def make_unit(h, g, bufs):
                # Chunk layout: non-diag js in threes (512 each, bank aligned),
                # then the diag chunk packed 512+384+128 | 256 into 2.5 banks.
                # chunks: list of (pt_col0, [(j, qlo, off_in_chunk, w)], width)
                chunks = []
                col = 0
                nd = 4 * g  # non-diagonal k-blocks
                for i0 in range(0, nd, 3):
                    entries = [
                        (j, GW * g, (j - i0) * GW, GW)
                        for j in range(i0, min(i0 + 3, nd))
                    ]
                    cw = len(entries) * GW
                    chunks.append((col, entries, cw))
                    col += cw
                d0 = 4 * g
                entries = [
                    (d0, QB * d0, 0, 512),
                    (d0 + 1, QB * (d0 + 1), 512, 384),
                    (d0 + 3, QB * (d0 + 3), 896, 128),
                    (d0 + 2, QB * (d0 + 2), 1024, 256),
                ]
                chunks.append((col, entries, 1280))
                col += 1280
                offs = {}
                for col0, entries, _ in chunks:
                    for (j, qlo, off, _w) in entries:
                        offs[j] = (qlo, col0 + off)
                pt_sb = ptp.tile(
                    [128, 12 * GW + 1280], bf16, tag="pt", name=f"pt{h}g{g}"
                )
                return (h, g, bufs, pt_sb, offs, chunks)

            # Global clock-based pacing: emit exp chunks on ACT's schedule
            # (the bottleneck engine), fill PE's spare time from a queue of
            # pending PV work. Clocks in PE cycles @2.4GHz; ACT cycles
            # count double.
            pe_clock = 0.0
            act_clock = 0.0
            pvq = []  # list of (unit_idx, pe_cost, closure), FIFO
            qi = 0

            def drain_pv(upto_unit=None, clock_limit=None):
                nonlocal qi, pe_clock
                while qi < len(pvq):
                    uidx, cost, fn = pvq[qi]
                    if upto_unit is not None and uidx > upto_unit:
                        break
                    if clock_limit is not None and pe_clock >= clock_limit:
                        break
                    fn()
                    pe_clock += cost
                    qi += 1

            head_bufs = [None] * HPC
            head_bufs[0] = load_head(0)
            uidx = 0
            for h in range(HPC):
                if h + 1 < HPC:
                    head_bufs[h + 1] = load_head(h + 1)
                gs = range(NG - 1, -1, -1) if h == HPC - 1 else range(NG)
                for g in gs:
                    # pt pool has 2 slots: before unit uidx's first exp can
                    # run, unit uidx-2's PV (which reads the other slot's
                    # predecessor) must be fully emitted on PE's stream.
                    drain_pv(upto_unit=uidx - 2)
                    u = make_unit(h, g, head_bufs[h])
                    for mms, mmcost, ex, acost in s_chunks(u):
                        # give PE filler work until ACT needs this chunk
                        drain_pv(clock_limit=act_clock - mmcost)
                        for mm in mms:
                            mm()
                        pe_clock += mmcost
                        ex()
                        act_clock = max(act_clock, pe_clock) + acost
                    pvq.extend(
                        (uidx, cost, fn) for cost, fn in pv_steps(u)
                    )
                    uidx += 1
            drain_pv()

    nc.compile()
    return nc


def _prep_core(q, k, v):
    """q,k,v: [HPC, S, D] f32 for one core -> device input dict."""
    qt = np.ascontiguousarray(q.transpose(0, 2, 1)).astype(_BF16)
    kt = np.ascontiguousarray(k.transpose(0, 2, 1)).astype(_BF16)
    va = np.empty((HPC, S, VW), dtype=np.float32)
    va[:, :, :D] = v
    va[:, :, D] = 1.0
    # [HPC, S, VW] -> [HPC, 128, NJ*VW]  with [p, j*VW+c] = va[j*128+p, c]
    va = np.ascontiguousarray(
        va.reshape(HPC, NJ, QB, VW).transpose(0, 2, 1, 3)
    ).reshape(HPC, QB, NJ * VW).astype(_BF16)
    return {"qt": qt, "kt": kt, "va": va}


def _run(query, key, value, trace=False):
    from concourse import bass_utils

    if "nc" not in _CACHE:
        _CACHE["nc"] = _build()
    nc = _CACHE["nc"]

    q = np.asarray(query, dtype=np.float32).reshape(B * H, S, D)
    k = np.asarray(key, dtype=np.float32).reshape(B * H, S, D)
    v = np.asarray(value, dtype=np.float32).reshape(B * H, S, D)
    tri = np.triu(np.ones((128, 128), dtype=np.float32)).astype(_BF16)

    in_maps = []
    for c in range(N_CORES):
        sl = slice(c * HPC, (c + 1) * HPC)
        m = _prep_core(q[sl], k[sl], v[sl])
        m["tri"] = tri
        in_maps.append(m)

    res = bass_utils.run_bass_kernel_spmd(
        nc, in_maps, core_ids=list(range(N_CORES)), trace=trace
    )
    outs = [res.results[c]["out"] for c in range(N_CORES)]
    full = np.concatenate(outs, axis=0).reshape(B, H, S, D).astype(np.float32)
    return full, res


def kernel(query, key, value, mask=None):
    """Full inputs in, full output out. `mask` is the causal mask from
    setup_inputs (strictly-upper-triangular True = disallowed); causality is
    implemented structurally so the tensor itself is not consumed."""
    out, _ = _run(query, key, value, trace=False)
    return out


# revision 15
# speedup vs baseline: 1.0567x; 1.0567x over previous
"""Causal multi-head attention on 8 TRN2 NeuronCores.

Problem: B=4, H=16, S=2048, D=128 fp32 causal attention.
Sharding: batch*heads (64) split 8-per-core across the 8 cores; each core
computes its heads fully independently (no collectives).

Per-core kernel strategy (bf16 matmuls, f32 accumulation):
  - scores computed TRANSPOSED: S^T[k,q] = K_j @ Q^T per (k-block j of 128,
    q-group g of 512), causal blocks only
  - exp on ScalarE (PSUM -> SBUF bf16), 1/sqrt(D) folded into activation
    scale; diagonal 128x128 blocks masked with a constant triangular tile
  - PV: out[q,:] = P^T_slice.T @ [V_j | ones]; the appended ones column
    yields the softmax denominator in the same accumulation. Output lands
    directly in [q, d] layout.
  - normalize with VectorE reciprocal + per-partition tensor_scalar mult
"""

import numpy as np
import ml_dtypes

B, H, S, D = 4, 16, 2048, 128
N_CORES = 8
HPC = (B * H) // N_CORES  # heads per core = 8
QB = 128                  # q/k block
GW = 512                  # q group width
NG = S // GW              # 4 groups per head
NJ = S // QB              # 16 k blocks
VW = D + 1                # V augmented with ones column = 129
SCALE = 1.0 / float(np.sqrt(D))

_BF16 = ml_dtypes.bfloat16

_CACHE = {}


def _build():
    import concourse.bass as bass  # noqa: F401
    import concourse.mybir as mybir
    from concourse import bacc
    from concourse.tile import TileContext

    f32 = mybir.dt.float32
    bf16 = mybir.dt.bfloat16
    EXP = mybir.ActivationFunctionType.Exp

    nc = bacc.Bacc("TRN2", target_bir_lowering=False, num_devices=N_CORES)

    qt_d = nc.dram_tensor("qt", [HPC, 128, S], bf16, kind="ExternalInput").ap()
    kt_d = nc.dram_tensor("kt", [HPC, 128, S], bf16, kind="ExternalInput").ap()
    va_d = nc.dram_tensor("va", [HPC, 128, NJ * VW], bf16, kind="ExternalInput").ap()
    tri_d = nc.dram_tensor("tri", [128, 128], bf16, kind="ExternalInput").ap()
    out_d = nc.dram_tensor("out", [HPC, S, D], f32, kind="ExternalOutput").ap()

    with TileContext(nc) as tc:
        with (
            tc.tile_pool(name="consts", bufs=1) as consts,
            tc.tile_pool(name="io", bufs=2) as io,
            tc.tile_pool(name="pt", bufs=3) as ptp,
            tc.tile_pool(name="ob", bufs=3) as obp,
            tc.tile_pool(name="rr", bufs=4) as rrp,
            tc.tile_pool(name="st", bufs=2, space="PSUM") as stp,
            tc.tile_pool(name="acc", bufs=2, space="PSUM") as accp,
        ):
            tri_sb = consts.tile([128, 128], bf16)

            def load_head(h):
                qt_sb = io.tile([128, S], bf16, tag="qt", name=f"qt{h}")
                kt_sb = io.tile([128, S], bf16, tag="kt", name=f"kt{h}")
                va_sb = io.tile([128, NJ * VW], bf16, tag="va", name=f"va{h}")
                m = (NJ * VW) // 2
                nc.sync.dma_start(out=qt_sb[:, 0:GW], in_=qt_d[h, :, 0:GW])
                nc.sync.dma_start(out=kt_sb[:, 0:GW], in_=kt_d[h, :, 0:GW])
                if h == 0:
                    nc.sync.dma_start(out=tri_sb[:, :], in_=tri_d[:, :])
                for p in range(1, 4):
                    lo, hi = p * GW, (p + 1) * GW
                    nc.sync.dma_start(out=qt_sb[:, lo:hi], in_=qt_d[h, :, lo:hi])
                    nc.sync.dma_start(out=kt_sb[:, lo:hi], in_=kt_d[h, :, lo:hi])
                nc.sync.dma_start(out=va_sb[:, 0:m], in_=va_d[h, :, 0:m])
                nc.sync.dma_start(out=va_sb[:, m:], in_=va_d[h, :, m:])
                return qt_sb, kt_sb, va_sb

            def s_chunks(u):
                """Per chunk of unit u: (mm_closures_with_cost, exp_closure,
                act_cost). S^T matmuls land in bank-aligned PSUM chunks
                (<=3 banks), one exp per chunk, diag masks after the exp."""
                h, g, bufs, pt_sb, offs, chunks = u
                qt_sb, kt_sb, _ = bufs
                qhi = GW * (g + 1)
                for ci, (col0, entries, cw) in enumerate(chunks):
                    st = stp.tile([128, 1536], f32, tag="st",
                                  name=f"st{h}g{g}c{ci}")
                    mms = []
                    mmcost = 0
                    for (j, qlo, off, w) in entries:
                        def mm(j=j, qlo=qlo, off=off, w=w, st=st):
                            nc.tensor.matmul(
                                st[:, off:off + w],
                                lhsT=kt_sb[:, j * QB:(j + 1) * QB],
                                rhs=qt_sb[:, qlo:qhi],
                                start=True, stop=True,
                            )
                        mms.append(mm)
                        mmcost += w + 8

                    def ex(col0=col0, cw=cw, st=st, entries=entries):
                        nc.scalar.activation(
                            pt_sb[:, col0:col0 + cw], st[:, 0:cw], EXP,
                            scale=SCALE,
                        )
                        for (j, qlo, off, w) in entries:
                            if j >= 4 * g:  # diag block: zero where k > q
                                nc.vector.tensor_mul(
                                    pt_sb[:, col0 + off:col0 + off + QB],
                                    pt_sb[:, col0 + off:col0 + off + QB],
                                    tri_sb[:, :],
                                )
                    yield mms, mmcost, ex, 2 * (cw + 222)

            def pv_steps(u):
                """(pe_cost, closure) steps: PV accumulation matmuls +
                normalize + store for unit u."""
                h, g, bufs, pt_sb, offs, _chunks = u
                _, _, va_sb = bufs
                o_sb = obp.tile([128, GW], f32, tag="ob", name=f"ob{h}g{g}")
                for c in range(4):
                    Q = 4 * g + c
                    qlo_c = GW * g + QB * c
                    acc = accp.tile([128, VW], f32, tag="acc", name=f"acc{h}g{g}c{c}")
                    for j in range(Q + 1):
                        qlo_j, col_j = offs[j]
                        off = col_j + (qlo_c - qlo_j)

                        def step(j=j, Q=Q, off=off, acc=acc):
                            nc.tensor.matmul(
                                acc[:, :],
                                lhsT=pt_sb[:, off:off + QB],
                                rhs=va_sb[:, j * VW:(j + 1) * VW],
                                start=(j == 0), stop=(j == Q),
                            )
                        yield 135, step

                    def fin(c=c, acc=acc, o_sb=o_sb):
                        r = rrp.tile([128, 1], f32, tag="r", name=f"r{h}g{g}c{c}")
                        nc.vector.reciprocal(r[:, :], acc[:, D:D + 1])
                        nc.vector.tensor_scalar_mul(
                            o_sb[:, c * QB:(c + 1) * QB], acc[:, 0:D], r[:, :]
                        )
                        if c == 3:
                            dst = out_d[h, GW * g:GW * (g + 1), :].rearrange(
                                "(c p) d -> p c d", p=128
                            )
                            src = o_sb[:, :].rearrange("p (c d) -> p c d", c=4)
                            nc.sync.dma_start(out=dst, in_=src)
                    yield 0, fin

            def make_unit(h, g, bufs):
                # Chunk layout: non-diag js in threes (512 each, bank aligned),
                # then the diag chunk packed 512+384+128 | 256 into 2.5 banks.
                # chunks: list of (pt_col0, [(j, qlo, off_in_chunk, w)], width)
                chunks = []
                col = 0
                nd = 4 * g  # non-diagonal k-blocks
                for i0 in range(0, nd, 3):
                    entries = [
                        (j, GW * g, (j - i0) * GW, GW)
                        for j in range(i0, min(i0 + 3, nd))
                    ]
                    cw = len(entries) * GW
                    chunks.append((col, entries, cw))
                    col += cw
                d0 = 4 * g
                entries = [
                    (d0, QB * d0, 0, 512),
                    (d0 + 1, QB * (d0 + 1), 512, 384),
                    (d0 + 3, QB * (d0 + 3), 896, 128),
                    (d0 + 2, QB * (d0 + 2), 1024, 256),
                ]
                chunks.append((col, entries, 1280))
                col += 1280
                offs = {}
                for col0, entries, _ in chunks:
                    for (j, qlo, off, _w) in entries:
                        offs[j] = (qlo, col0 + off)
                pt_sb = ptp.tile(
                    [128, 12 * GW + 1280], bf16, tag="pt", name=f"pt{h}g{g}"
                )
                return (h, g, bufs, pt_sb, offs, chunks)

            # Global clock-based pacing: emit exp chunks on ACT's schedule
            # (the bottleneck engine), fill PE's spare time from a queue of
            # pending PV work. Clocks in PE cycles @2.4GHz; ACT cycles
            # count double.
            pe_clock = 0.0
            act_clock = 0.0
            pvq = []  # list of (unit_idx, pe_cost, closure), FIFO
            qi = 0

            def drain_pv(upto_unit=None, clock_limit=None):
                nonlocal qi, pe_clock
                while qi < len(pvq):
                    uidx, cost, fn = pvq[qi]
                    if upto_unit is not None and uidx > upto_unit:
                        break
                    if clock_limit is not None and pe_clock >= clock_limit:
                        break
                    fn()
                    pe_clock += cost
                    qi += 1

            head_bufs = [None] * HPC
            head_bufs[0] = load_head(0)
            uidx = 0
            for h in range(HPC):
                if h + 1 < HPC:
                    head_bufs[h + 1] = load_head(h + 1)
                gs = range(NG - 1, -1, -1) if h == HPC - 1 else range(NG)
                for g in gs:
                    # pt pool has 3 slots: before unit uidx's first exp can
                    # run, unit uidx-3's PV (the slot's previous holder's
                    # reader) must be fully emitted on PE's stream.
                    drain_pv(upto_unit=uidx - 3)
                    u = make_unit(h, g, head_bufs[h])
                    for mms, mmcost, ex, acost in s_chunks(u):
                        # give PE filler work until ACT needs this chunk
                        drain_pv(clock_limit=act_clock - mmcost)
                        for mm in mms:
                            mm()
                        pe_clock += mmcost
                        ex()
                        act_clock = max(act_clock, pe_clock) + acost
                    pvq.extend(
                        (uidx, cost, fn) for cost, fn in pv_steps(u)
                    )
                    uidx += 1
            drain_pv()

    nc.compile()
    return nc


def _prep_core(q, k, v):
    """q,k,v: [HPC, S, D] f32 for one core -> device input dict."""
    qt = np.ascontiguousarray(q.transpose(0, 2, 1)).astype(_BF16)
    kt = np.ascontiguousarray(k.transpose(0, 2, 1)).astype(_BF16)
    va = np.empty((HPC, S, VW), dtype=np.float32)
    va[:, :, :D] = v
    va[:, :, D] = 1.0
    # [HPC, S, VW] -> [HPC, 128, NJ*VW]  with [p, j*VW+c] = va[j*128+p, c]
    va = np.ascontiguousarray(
        va.reshape(HPC, NJ, QB, VW).transpose(0, 2, 1, 3)
    ).reshape(HPC, QB, NJ * VW).astype(_BF16)
    return {"qt": qt, "kt": kt, "va": va}


def _run(query, key, value, trace=False):
    from concourse import bass_utils

    if "nc" not in _CACHE:
        _CACHE["nc"] = _build()
    nc = _CACHE["nc"]

    q = np.asarray(query, dtype=np.float32).reshape(B * H, S, D)
    k = np.asarray(key, dtype=np.float32).reshape(B * H, S, D)
    v = np.asarray(value, dtype=np.float32).reshape(B * H, S, D)
    tri = np.triu(np.ones((128, 128), dtype=np.float32)).astype(_BF16)

    in_maps = []
    for c in range(N_CORES):
        sl = slice(c * HPC, (c + 1) * HPC)
        m = _prep_core(q[sl], k[sl], v[sl])
        m["tri"] = tri
        in_maps.append(m)

    res = bass_utils.run_bass_kernel_spmd(
        nc, in_maps, core_ids=list(range(N_CORES)), trace=trace
    )
    outs = [res.results[c]["out"] for c in range(N_CORES)]
    full = np.concatenate(outs, axis=0).reshape(B, H, S, D).astype(np.float32)
    return full, res


def kernel(query, key, value, mask=None):
    """Full inputs in, full output out. `mask` is the causal mask from
    setup_inputs (strictly-upper-triangular True = disallowed); causality is
    implemented structurally so the tensor itself is not consumed."""
    out, _ = _run(query, key, value, trace=False)
    return out
